# revision 1
# baseline (speedup 1.0000x reference)
"""Trainium2 Bass kernel for nn_MaskGen: per-sample 1x1 conv (channel dot)
+ global BatchNorm2d(1) (training-mode batch stats) + LeakyReLU(0.1).

Sharding: pure data parallel over batch B=32 -> 4 batches per core on 8 cores.
Global batch-norm stats via a tiny padded [1,8] AllReduce inside the kernel.

Per core:
  - feats shard viewed as [256, 25600] (row b*64+c), split into 2 "groups"
    of 2 batches (128 rows = 2 batches x 64 channels on partitions).
  - Matmul with feats as the STATIONARY side: lhsT = feats chunk [128, 128hw],
    rhs = block-diagonal sf [128, 2] (sf for the 2 batches of the group on
    disjoint 64-row halves).  out = [128 hw-partitions, 2 batches]; the
    group's mask accumulates as [128, 400] (col 2*ch + r, partition = hw %
    128) in a single PSUM bank, where it stays until the final normalize.
  - feats tiles stream via HWDGE (nc.sync) in 10 x 1.31MB loads; matmuls
    chase the loads so PE stays warm and DMA stays at the HBM roofline.
  - Stats: per-partition sumsq via ACT Square accum_out, per-partition sum
    via DVE tensor_reduce (two engines in parallel, straight from PSUM),
    groups combined on DVE, partition-reduced AND broadcast by a
    ones-matmul, then an 8-core AllReduce of one padded 32B row.
  - Normalize: y = mask*scale + shift read directly from PSUM (ACT Identity
    with per-partition scale/bias APs for group 0, DVE tensor_scalar for
    group 1), LeakyReLU as max(y, 0.1*y) on DVE, one output DMA per group
    in a permuted layout the host un-permutes during unshard.

Sync-capacity constraints (walrus codegen): DMA instructions carry at most
ONE semaphore wait; _split_multi_waits hoists any extras onto standalone
EventSemaphore instructions as a safety net.
"""

import os
from contextlib import ExitStack

import numpy as np

import concourse.bass as bass
import concourse.tile as tile
from concourse import library_config, mybir
from concourse.bass_utils import run_bass_kernel_spmd

N_CORES = 8
B, C, H, W = 32, 64, 160, 160
HW = H * W                # 25600
BPC = B // N_CORES        # 4 batches per core
NG = BPC // 2             # 2 groups (pairs of batches) per core
ROWS = BPC * C            # 256 feats rows per core
N_TOT = B * HW            # 819200 elements in the batchnorm stats
P = 128                   # hw elements per matmul chunk (PE stationary cols)
NCHUNK = HW // P          # 200 chunks per group
TILE_W = 2560             # feats DMA tile width (655 KB per load)
NLOAD = HW // TILE_W      # 5 loads per group
MM_PER_LOAD = TILE_W // P  # 40 matmuls per loaded tile
EPS = 1e-5
SLOPE = 0.1

F32 = mybir.dt.float32
IN_DT = mybir.dt.bfloat16
IN_DT_NP = np.dtype(mybir.dt.np(mybir.dt.bfloat16))


def _body(ctx: ExitStack, tc: "tile.TileContext", feats, sf, bnwb, out):
    nc = tc.nc
    AF = mybir.ActivationFunctionType
    ALU = mybir.AluOpType

    singles = ctx.enter_context(tc.tile_pool(name="singles", bufs=1))
    # one slot per feats tile: no slot reuse -> feats DMAs carry no WAR wait
    ftp = ctx.enter_context(tc.tile_pool(name="ftp", bufs=NG * NLOAD))
    psum = ctx.enter_context(tc.tile_pool(name="psum", bufs=1, space="PSUM"))
    work = ctx.enter_context(tc.tile_pool(name="work", bufs=2))
    norm = ctx.enter_context(tc.tile_pool(name="norm", bufs=3))
    dram = ctx.enter_context(tc.tile_pool(name="dram", bufs=1, space="DRAM"))

    # --- block-diagonal sf weights (host-precomputed): col 2g+r holds
    #     sf[2g+r,:] in rows 64r:64r+64, zeros elsewhere.
    w_sb = singles.tile([128, 2 * NG], IN_DT)
    nc.sync.dma_start(out=w_sb, in_=sf)

    # ones for the partition-reduce + broadcast matmul
    ones_sb = singles.tile([128, 128], F32)
    nc.vector.memset(ones_sb, 1.0)

    # bn weight+bias broadcast to all partitions: [128, 2] = [w, b]
    wbb = singles.tile([128, 2], F32, tag="wbb")
    nc.scalar.dma_start(out=wbb, in_=bnwb.to_broadcast([128, 2]))

    eps_sb = singles.tile([128, 1], F32, tag="eps_sb")
    nc.vector.memset(eps_sb, EPS)

    # padded 32-byte collective payload row: [sum, sumsq, 0...]
    cc_src = singles.tile([1, 8], F32, tag="cc_src")
    nc.vector.memset(cc_src, 0.0)

    # PE warm-up dummies: absorb the w_sb-DMA and ones-memset waits into
    # PE's vector clock so no later matmul needs a second wait slot.
    warm_ps = psum.tile([128, 1], F32, tag="warm")
    nc.tensor.matmul(out=warm_ps[: 2 * NG, :], lhsT=w_sb, rhs=w_sb[:, 0:1],
                     start=True, stop=True)
    nc.tensor.matmul(out=warm_ps, lhsT=ones_sb, rhs=ones_sb[:, 0:1],
                     start=True, stop=True)

    # --- channel-dot matmuls, feats stationary, mask accumulates in PSUM
    sums = singles.tile([128, NG], F32, tag="sums")  # DVE-written
    sqs = singles.tile([128, NG], F32, tag="sqs")    # ACT-written
    mask_ps = []
    for g in range(NG):
        mp = psum.tile([128, 2 * NCHUNK], F32, tag=f"mask{g}")
        mask_ps.append(mp)
        for l in range(NLOAD):
            ft = ftp.tile([128, TILE_W], IN_DT, tag="ft")
            nc.sync.dma_start(
                out=ft,
                in_=feats[128 * g : 128 * (g + 1), TILE_W * l : TILE_W * (l + 1)],
            )
            for m in range(MM_PER_LOAD):
                ch = MM_PER_LOAD * l + m
                nc.tensor.matmul(
                    out=mp[:, 2 * ch : 2 * ch + 2],
                    lhsT=ft[:, P * m : P * (m + 1)],
                    rhs=w_sb[:, 2 * g : 2 * g + 2],
                    start=True,
                    stop=True,
                )
        # group stats straight from PSUM: sumsq on ACT, sum on DVE
        sq = work.tile([128, 2 * NCHUNK], F32, tag="sq")
        nc.scalar.activation(
            out=sq,
            in_=mp,
            func=AF.Square,
            accum_out=sqs[:, g : g + 1],
        )
        nc.vector.tensor_reduce(
            out=sums[:, g : g + 1],
            in_=mp,
            axis=mybir.AxisListType.X,
            op=ALU.add,
        )

    # combine groups per partition: [sum, sumsq] on each partition
    pp2 = singles.tile([128, 2], F32, tag="pp2")
    nc.vector.tensor_add(out=pp2[:, 0:1], in0=sums[:, 0:1], in1=sums[:, 1:2])
    nc.vector.tensor_add(out=pp2[:, 1:2], in0=sqs[:, 0:1], in1=sqs[:, 1:2])

    # partition-reduce AND broadcast: stats_ps[m, j] = sum_p pp2[p, j]
    stats_ps = psum.tile([128, 2], F32, tag="stats")
    nc.tensor.matmul(out=stats_ps, lhsT=ones_sb, rhs=pp2, start=True, stop=True)
    nc.vector.tensor_copy(out=cc_src[:, 0:2], in_=stats_ps[0:1, :])

    # --- AllGather one padded 32B row per core (cheaper than AllReduce:
    # plain copy chunks, no CCE reduce reads); the cross-core sum happens
    # on-core below.
    cc_in = dram.tile([1, 8], F32, tag="cc_in")
    cc_out = dram.tile([1, 8 * N_CORES], F32, tag="cc_out")
    nc.scalar.dma_start(out=cc_in[:], in_=cc_src)
    nc.gpsimd.collective_compute(
        "AllGather",
        mybir.AluOpType.bypass,
        replica_groups=[list(range(N_CORES))],
        ins=[cc_in.opt()],
        outs=[cc_out.opt()],
    )
    # gathered rows back as one 256B row, partition-broadcast via a K=1
    # matmul (avoids the slower DRE-replication DMA descriptor pattern).
    allred_sb = singles.tile([1, 8 * N_CORES], F32, tag="allred_sb")
    nc.scalar.dma_start(out=allred_sb, in_=cc_out[:])
    stats_bc = psum.tile([128, 8 * N_CORES], F32, tag="stats_bc")
    nc.tensor.matmul(out=stats_bc, lhsT=ones_sb[0:1, :], rhs=allred_sb,
                     start=True, stop=True)
    # sum the 8 per-rank [sum, sumsq] pairs: view [128, (rank, col)] as
    # [128, col, rank] and reduce the innermost rank axis.
    totals = singles.tile([128, 2], F32, tag="totals")
    nc.vector.tensor_reduce(
        out=totals,
        in_=stats_bc[:, 0 : 8 * N_CORES].rearrange(
            "p (r c) -> p c r", r=N_CORES, c=8
        )[:, 0:2, :],
        axis=mybir.AxisListType.X,
        op=ALU.add,
    )

    # --- scalar math, replicated across partitions ([128,1] tiles)
    me2 = singles.tile([128, 2], F32, tag="me2")   # [mean, E[x^2]]
    nc.vector.tensor_scalar_mul(out=me2, in0=totals, scalar1=1.0 / N_TOT)
    msq = singles.tile([128, 1], F32, tag="msq")
    nc.vector.tensor_mul(out=msq, in0=me2[:, 0:1], in1=me2[:, 0:1])
    var = singles.tile([128, 1], F32, tag="var")
    nc.vector.tensor_sub(out=var, in0=me2[:, 1:2], in1=msq)
    std = singles.tile([128, 1], F32, tag="std")
    nc.scalar.activation(out=std, in_=var, func=AF.Sqrt, bias=eps_sb)
    inv = singles.tile([128, 1], F32, tag="inv")
    nc.vector.reciprocal(out=inv, in_=std)
    scl = singles.tile([128, 1], F32, tag="scl")
    nc.vector.tensor_mul(out=scl, in0=inv, in1=wbb[:, 0:1])
    msc = singles.tile([128, 1], F32, tag="msc")
    nc.vector.tensor_mul(out=msc, in0=me2[:, 0:1], in1=scl)
    shf = singles.tile([128, 1], F32, tag="shf")
    nc.vector.tensor_sub(out=shf, in0=wbb[:, 1:2], in1=msc)

    # --- normalize + LeakyReLU + store (permuted layout, host un-permutes)
    # mask layout: mp[p, 2*ch + r] = mask[2g+r, 128*ch + p]
    # group 0 affine on ACT (reads PSUM), group 1 affine on DVE.
    y0 = norm.tile([128, 2 * NCHUNK], F32, tag="y0")
    nc.scalar.activation(out=y0, in_=mask_ps[0], func=AF.Identity,
                         bias=shf, scale=scl)
    o0 = norm.tile([128, 2 * NCHUNK], IN_DT, tag="o0")
    nc.vector.scalar_tensor_tensor(
        out=o0, in0=y0, scalar=SLOPE, in1=y0, op0=ALU.mult, op1=ALU.max
    )
    nc.sync.dma_start(out=out[:, 0 : 2 * NCHUNK], in_=o0)

    y1 = norm.tile([128, 2 * NCHUNK], F32, tag="y1")
    nc.vector.tensor_scalar(
        out=y1, in0=mask_ps[1], scalar1=scl, scalar2=shf,
        op0=ALU.mult, op1=ALU.add,
    )
    o1 = norm.tile([128, 2 * NCHUNK], IN_DT, tag="o1")
    nc.vector.scalar_tensor_tensor(
        out=o1, in0=y1, scalar=SLOPE, in1=y1, op0=ALU.mult, op1=ALU.max
    )
    # group-1 store on the ACT HWDGE ring so both output DMAs dispatch in
    # parallel with group-0's on the SP ring.
    nc.scalar.dma_start(out=out[:, 2 * NCHUNK : 4 * NCHUNK], in_=o1)


_PATCH = {}


def _patch_mailbox_wait(nc):
    """Raise the mailbox wait from the scheduling-safe 0 to the real 14."""
    name = _PATCH.pop("mbx_wait_name")
    n = 0
    for fn in nc.m.functions:
        for bb in fn.blocks:
            for inst in bb.instructions:
                if inst.name == name:
                    si = inst.sync_info
                    assert si is not None and len(si.on_wait) >= 1, si
                    for w in si.on_wait:
                        if w.ant_name == "mbx_sem":
                            w.wait_value = 14
                            n += 1
    assert n == 1, f"mailbox wait patch applied {n} times"


def _split_multi_waits(nc):
    """walrus codegen accepts one semaphore wait per instruction (each ISA
    struct embeds a single EVENTS slot).  Tile's scheduler attaches several;
    hoist all but the last onto standalone EventSemaphore instructions on the
    same engine, immediately before the original instruction."""
    n = 0
    for fn in nc.m.functions:
        for bb in fn.blocks:
            insts = list(bb.instructions)
            if not any(
                i.sync_info is not None and len(i.sync_info.on_wait) > 1
                for i in insts
            ):
                continue
            new_insts = []
            for inst in insts:
                si = inst.sync_info
                if si is not None and len(si.on_wait) > 1:
                    waits = list(si.on_wait)
                    for w in waits[:-1]:
                        n += 1
                        ev = mybir.InstEventSemaphore(
                            name=f"{inst.name}-sw{n}",
                            ins=[],
                            outs=[],
                            sync_info=mybir.SyncInfo(on_wait=[w], on_update=[]),
                        )
                        ev.engine = inst.engine
                        nc.register_instruction(ev, overwrite=True)
                        new_insts.append(ev)
                    si.on_wait = [waits[-1]]
                new_insts.append(inst)
            bb.instructions = new_insts
    return n


def build_nc():
    nc = bass.Bass(num_devices=N_CORES)
    feats = nc.declare_dram_parameter("feats", [ROWS, HW], IN_DT, isOutput=False)
    sf = nc.declare_dram_parameter("sf", [128, 2 * NG], IN_DT, isOutput=False)
    bnwb = nc.declare_dram_parameter("bn_wb", [1, 2], F32, isOutput=False)
    out = nc.declare_dram_parameter("out", [128, 2 * NG * NCHUNK], IN_DT, isOutput=True)
    with tile.TileContext(nc, num_cores=N_CORES) as tc:
        with ExitStack() as ctx:
            _body(ctx, tc, feats[:], sf[:], bnwb[:], out[:])
    _split_multi_waits(nc)
    return nc


def make_in_maps(sf, feats, bn_weight, bn_bias):
    sf = np.asarray(sf)
    feats = np.asarray(feats)
    bnwb = np.array(
        [[np.float32(np.asarray(bn_weight).reshape(-1)[0]),
          np.float32(np.asarray(bn_bias).reshape(-1)[0])]],
        dtype=np.float32,
    )
    sf2 = np.ascontiguousarray(sf.reshape(B, C)).astype(IN_DT_NP)
    in_maps = []
    for k in range(N_CORES):
        fshard = np.ascontiguousarray(
            feats[BPC * k : BPC * (k + 1)].reshape(ROWS, HW)
        ).astype(IN_DT_NP)
        wmat = np.zeros((128, 2 * NG), dtype=IN_DT_NP)
        for g in range(NG):
            for r in range(2):
                wmat[64 * r : 64 * r + 64, 2 * g + r] = sf2[BPC * k + 2 * g + r]
        in_maps.append(
            {
                "feats": fshard,
                "sf": wmat,
                "bn_wb": bnwb,
            }
        )
    return in_maps


_NC_CACHE = {}


def get_nc():
    if "nc" not in _NC_CACHE:
        _NC_CACHE["nc"] = build_nc()
    return _NC_CACHE["nc"]


def assemble(results):
    parts = []
    for r in results:
        a = np.asarray(r["out"], dtype=np.float32).reshape(128, NG, NCHUNK, 2)
        # [p, g, ch, r] -> [g, r, ch, p] -> [BPC, HW]
        parts.append(np.ascontiguousarray(a.transpose(1, 3, 2, 0)).reshape(BPC, HW))
    return np.concatenate(parts, axis=0).reshape(B, 1, H, W).astype(np.float32)


def kernel(sf, feats, bn_weight, bn_bias):
    nc = get_nc()
    in_maps = make_in_maps(sf, feats, bn_weight, bn_bias)
    res = run_bass_kernel_spmd(nc, in_maps, list(range(N_CORES)))
    return assemble(res.results)

